# revision 29
# baseline (speedup 1.0000x reference)
"""Distributed Trainium2 (Bass/Tile) kernel for a batched quantized matmul.

Reference computation (all shapes hardcoded):
    out[s,b,m,n] = sum_k (x[s,b,m,k] + 66)*0.03 * (y[b,k,n] - 160)*0.025
    x: [7, 8, 1024, 1024] f32 holding ints in [-128, 127]
    y: [8, 1024, 1024]    f32 holding ints in [0, 255]
    out: [7, 8, 1024, 1024] f32

Sharding: data-parallel over B=8 -> one batch element b per NeuronCore.
Core b gets x[:, b] and y[b]; no collectives needed.

Device kernel (per core), fp8 DoubleRow variant:
  - The rel-err gate is 2e-2; quantizing the zero-point-shifted operands
    (x+66 in [-62,193], y-160 in [-160,95]) to TRN fp8e4 (e4m3, max 240)
    costs 4.8e-3 rel err (validated in numpy AND on hw) -- well inside
    the gate. Host pre-applies the zero points during the fp8 cast, so
    the device does no dequant arithmetic at all; the combined scale
    0.03*0.025 = 7.5e-4 is fused into the PSUM->SBUF eviction.
  - fp8e4 matmuls in DoubleRow mode contract 256 k-elements per
    instruction (2 multiplies/cell/cycle): half the bf16 instruction
    count for the same work. 448 MMs x 213ns = 95.5us PE floor; the
    kernel streams them back-to-back at that rate (measured).
  - Plain DoubleRow ran MMs at 259ns: the 256-column non-contiguous
    LDWEIGHTS stole the rhs stream's SBUF/XBUS bandwidth. With
    DoubleRowSwInterleave the host pre-interleaves each weight tile
    into one contiguous 256B/partition block; LDWEIGHTS (130ns) then
    overlaps 100% and MMs hit the 213ns roofline. The y tiles must
    stay [128, 2, 1024] (i-row stride 1024B): per-(ki,nj) half tiles
    throttle every MM to 259ns (measured 20us slower).
  - Startup (trace-measured): the runtime preamble gates the first DMA
    issue to ~7.3us; the PE HAM clock gate holds 1.2GHz until its
    trailing activity window fills, and every PE idle gap pushes the
    2.4GHz ramp later. Dummy warm-up matmuls keep PE busy from the
    preamble to the first operand arrival (ramp ~11.5us vs ~17us with
    a gappy start). The first 1.5MB of operands are bandwidth-bound:
    y rides the sync HWDGE ring, the first head x chunk rides the
    scalar HWDGE ring (prompt completion events; its dma_start
    precedes the act-table load and first eviction in program order),
    and the rest of x rides the gpsimd SWDGE ring -- HWDGE+SWDGE pull
    in parallel, while two HWDGE rings would share one ~180GB/s
    budget (measured 4us slower). SWDGE completion EVENTS lag ~2-3us
    while the gpsimd sequencer still has dma_start issues queued, so
    the gpsimd descriptor list is kept short: 3 head chunks, ONE
    640KB tail transfer (p-major host layout), one 1MB descriptor per
    s>=1, with s>=3 additionally held back by pool recycling.
  - The s=0 head runs 4 output stripes ki-outer so each 128KB head
    chunk feeds ~1.7us of matmuls at the chunk arrival cadence.
  - Eviction alternates ScalarE/DVE per stripe and store issues ride
    the sync queue: one queue cannot hold 57 x 1.26us evictions plus
    57 x 0.7us dma_start issue slots inside the PE span.
  - Output is stored bf16 (halves out-DMA; +2e-4 rel err) and upcast
    to f32 on the host.
  - Tail: the final stripe's two half-evictions drain on both evictor
    engines and their store issues ride different queues (scalar +
    sync) so they don't serialize after the last matmul.
"""

import numpy as np
import ml_dtypes

import concourse.bass as bass
import concourse.mybir as mybir
from concourse import bacc
from concourse.tile import TileContext
from concourse.bass_utils import run_bass_kernel_spmd

S, B, M, K, N = 7, 8, 1024, 1024, 1024
P = 128          # SBUF partitions / PE array dim
NB = 512         # one PSUM bank of fp32
KP = 2 * P       # k-elements contracted per DoubleRow matmul
KTT, MTT = K // KP, M // P  # 4, 8 (host-side tiling of the x layout)
MJH = 4          # head stripes (s=0 warm-up path)
X_ZP = -66.0
Y_ZP = 160.0
OUT_SCALE = 0.03 * 0.025
BF16 = mybir.dt.bfloat16
FP8 = mybir.dt.float8e4
F32 = mybir.dt.float32
ACT_COPY = mybir.ActivationFunctionType.Copy
DR_SW = mybir.MatmulPerfMode.DoubleRowSwInterleave

_CACHED_NC = None


def build():
    # Bacc (not plain Bass): its finalize() runs generate_event_semaphores,
    # which splits multi-wait sync_info to the <=1-wait-per-instruction HW
    # limit (walrus rejects the unsplit form with "Too many sync waits").
    nc = bacc.Bacc("TRN2", target_bir_lowering=False)
    KT, MT, NT = K // KP, M // P, N // NB  # 4, 8, 2
    MR = MT - MJH  # tail stripes of s=0
    # x weight layout (DoubleRowSwInterleave, see _shard_inputs):
    # within a (ki2, mj) block of 256: position 2*j + i holds column
    # (127 - j) of k-subtile i. Three tensors so each startup DMA is a
    # single contiguous 2D transfer:
    #   x0h[ki, p, mj*256+c]           s=0, head stripes mj<MJH
    #   x0t[p, ki*MR*256 + mj'*256+c]  s=0, tail stripes (p-major: ONE DMA)
    #   xs[s-1, p, ki*MT*256 + mj*256+c]  s>=1 (p-major: one DMA per s)
    x0h_d = nc.declare_dram_parameter("x0h", [KT, P, MJH * 2 * P], FP8,
                                      isOutput=False)
    x0t_d = nc.declare_dram_parameter("x0t", [P, KT * MR * 2 * P], FP8,
                                      isOutput=False)
    xs_d = nc.declare_dram_parameter("xs", [S - 1, P, KT * MT * 2 * P], FP8,
                                     isOutput=False)
    # y pre-tiled per ki2: y_d[ki2, p, i, n] = yq[ki2*256+i*128+p, n]
    y_d = nc.declare_dram_parameter("y", [KT, P, 2, N], FP8, isOutput=False)
    o_d = nc.declare_dram_parameter("out", [S, M, N], BF16, isOutput=True)

    with TileContext(nc) as tc:
        with tc.tile_pool(name="ypool", bufs=1) as ypool, \
             tc.tile_pool(name="hpool", bufs=KT) as hpool, \
             tc.tile_pool(name="tpool", bufs=1) as tpool, \
             tc.tile_pool(name="xpool", bufs=2) as xpool, \
             tc.tile_pool(name="pspool", bufs=4, space="PSUM") as pspool, \
             tc.tile_pool(name="opool", bufs=6) as opool:
            # Warm-up: dummy matmuls keep PE busy from the preamble to the
            # first operand arrival (HAM clock ramp; see module docstring).
            # Only one column is memset (tile allocation needs a producer);
            # the rest is read as garbage, which is fine: the PE has no
            # traps, the warm PSUM bank is never read, and the first real
            # matmul's start=True resets it.
            warm_src = ypool.tile([P, NB], BF16, tag="warmsrc")
            nc.vector.memset(warm_src[:, 0:1], 1.0)
            warm_ps = pspool.tile([P, N], F32, tag="ps", name="warm")
            for _ in range(7):
                nc.tensor.matmul(warm_ps[:, 0:NB], warm_src[:, 0:P],
                                 warm_src[:], start=True, stop=True)
            nc.tensor.matmul(warm_ps[:, 0:NB // 2], warm_src[:, 0:P],
                             warm_src[:, 0:NB // 2], start=True, stop=True)

            # Startup loads (ring assignment rationale in module docstring).
            yq = [None] * KT
            xh = [None] * KT
            for ki in range(KT):
                xh[ki] = hpool.tile([P, MJH, 2 * P], FP8, tag="xh",
                                    name=f"xh{ki}")
                yq[ki] = ypool.tile([P, 2, N], FP8, tag=f"y{ki}",
                                    name=f"yt{ki}")
            xt0 = tpool.tile([P, KT, MR, 2 * P], FP8, tag="xt0")
            nc.scalar.dma_start(out=xh[0][:], in_=x0h_d[0])
            for ki in range(KT):
                nc.sync.dma_start(out=yq[ki][:], in_=y_d[ki])
            for ki in range(1, KT):
                nc.gpsimd.dma_start(out=xh[ki][:], in_=x0h_d[ki])
            nc.gpsimd.dma_start(out=xt0[:], in_=x0t_d[:])

            def evict(ot_sl, ps_sl, odd):
                # PSUM -> SBUF bf16 with fused scale, alternating between
                # the Scalar and Vector engines so neither eviction queue
                # accumulates backlog against the PE stream (a single queue
                # carrying all 57 x ~1.26us evictions plus issue overhead
                # runs within ~5% of the whole kernel span).
                if odd:
                    nc.vector.tensor_scalar_mul(ot_sl, ps_sl, OUT_SCALE)
                else:
                    nc.scalar.activation(ot_sl, ps_sl, ACT_COPY,
                                         scale=OUT_SCALE)

            def store(dram_sl, ot_sl, odd, queue=None):
                # store issues ride the near-idle sync queue: the ~0.7us
                # dma_start sequencer cost plus the ~0.75us cross-queue
                # wait fit easily there, and the store is off the
                # PSUM-recycle critical path (it only reads the SBUF copy)
                (queue or nc.sync).dma_start(out=dram_sl, in_=ot_sl)

            def mj_group(s, mj, lhsT_of, odd, split_evict=False):
                """One output stripe [128, 1024]: ki-inner accumulation into
                a 2-bank PSUM tile, then a single eviction + store. For the
                very last group, evict/store per nj half instead so the nj=0
                half drains while nj=1's final matmuls still stream."""
                pst = pspool.tile([P, N], F32, tag="ps", name="ps")
                ot = opool.tile([P, N], BF16, tag="o", name="ot")
                for ki in range(KT):
                    lhsT = lhsT_of(ki, mj)
                    for nj in range(NT):
                        nc.tensor.matmul(
                            pst[:, nj * NB:(nj + 1) * NB], lhsT,
                            yq[ki][:, :, nj * NB:(nj + 1) * NB],
                            start=(ki == 0), stop=(ki == KT - 1),
                            perf_mode=DR_SW)
                if split_evict:
                    # last stripe: drain the two nj halves on the two
                    # evictor queues in parallel, store issues on
                    # different queues so they don't serialize
                    for nj in range(NT):
                        sl = slice(nj * NB, (nj + 1) * NB)
                        evict(ot[:, sl], pst[:, sl], nj % 2)
                        store(o_d[s, mj * P:(mj + 1) * P, sl], ot[:, sl],
                              nj % 2,
                              queue=(nc.scalar if nj == 0 else nc.sync))
                else:
                    evict(ot[:], pst[:], odd)
                    store(o_d[s, mj * P:(mj + 1) * P, :], ot[:], odd)

            # s=0: head stripes ki-outer (consume each head chunk as it
            # lands), then the tail stripes from the single tail transfer.
            head = [pspool.tile([P, N], F32, tag="ps", name=f"ph{mj}")
                    for mj in range(MJH)]
            for ki in range(KT):
                for mj in range(MJH):
                    lhsT = xh[ki][:, mj, :]
                    for nj in range(NT):
                        nc.tensor.matmul(
                            head[mj][:, nj * NB:(nj + 1) * NB], lhsT,
                            yq[ki][:, :, nj * NB:(nj + 1) * NB],
                            start=(ki == 0), stop=(ki == KT - 1),
                            perf_mode=DR_SW)
            for mj in range(MJH):
                ot = opool.tile([P, N], BF16, tag="o", name="oth")
                evict(ot[:], head[mj][:], mj % 2)
                store(o_d[0, mj * P:(mj + 1) * P, :], ot[:], mj % 2)
            for mj in range(MJH, MT):
                mj_group(0, mj, lambda ki, mj: xt0[:, ki, mj - MJH, :],
                         mj % 2)

            # s>=1: one 1MB descriptor per s on the gpsimd ring; xpool
            # bufs=2 lets s+1 prefetch while s streams and holds s+2 back
            # until s's last LDWEIGHTS.
            for s in range(1, S):
                xb = xpool.tile([P, KT, MT, 2 * P], FP8, tag="xb")
                nc.gpsimd.dma_start(out=xb[:], in_=xs_d[s - 1])
                for mj in range(MT):
                    mj_group(s, mj, lambda ki, mj: xb[:, ki, mj, :],
                             mj % 2,
                             split_evict=(s == S - 1 and mj == MT - 1))
    nc.finalize()
    return nc


def _shard_inputs(x, y):
    f8 = ml_dtypes.float8_e4m3
    in_maps = []
    MR = MTT - MJH
    for b in range(B):
        # zero points pre-applied; |values| <= 193 fit e4m3 (max 240)
        # with <= 6.25% per-element rounding error -> ~4.6e-3 rel err.
        # x shard: k-major transpose, then the DoubleRowSwInterleave weight
        # layout (see build()): per (s, ki2, mj) block of 256, position
        # 2*j + i holds column (127 - j) of k-subtile i.
        xq = (np.ascontiguousarray(x[:, b].transpose(0, 2, 1))
              - np.float32(X_ZP)).astype(f8)          # [S, K, M]
        a = xq.reshape(S, KTT, 2, P, MTT, P)          # [s, ki2, i, p, mj, j]
        a = a.transpose(0, 1, 3, 4, 5, 2)[:, :, :, :, ::-1, :]
        a = np.ascontiguousarray(a).reshape(S, KTT, P, MTT * 2 * P)
        x0h = np.ascontiguousarray(a[0][:, :, :MJH * 2 * P])
        x0t = np.ascontiguousarray(
            a[0][:, :, MJH * 2 * P:].transpose(1, 0, 2)).reshape(
                P, KTT * MR * 2 * P)
        xs = np.ascontiguousarray(a[1:].transpose(0, 2, 1, 3)).reshape(
            S - 1, P, KTT * MTT * 2 * P)
        # y: per-ki2 DoubleRow tile layout [ki2, p, i, n] (one DMA per tile)
        yq = (y[b] - np.float32(Y_ZP)).astype(f8)    # [K, N]
        yq = yq.reshape(KTT, 2, P, N).transpose(0, 2, 1, 3)
        in_maps.append({
            "x0h": x0h,
            "x0t": x0t,
            "xs": xs,
            "y": np.ascontiguousarray(yq),
        })
    return in_maps


def run(x, y, trace=False):
    global _CACHED_NC
    if _CACHED_NC is None:
        _CACHED_NC = build()
    nc = _CACHED_NC
    in_maps = _shard_inputs(x, y)
    res = run_bass_kernel_spmd(nc, in_maps, core_ids=list(range(B)), trace=trace)
    out = np.stack([np.asarray(res.results[b]["out"]) for b in range(B)], axis=1)
    return out.astype(np.float32), res


def kernel(x, y):
    out, _ = run(x, y, trace=False)
    return out


# revision 30
# speedup vs baseline: 1.0051x; 1.0051x over previous
"""Distributed Trainium2 (Bass/Tile) kernel for a batched quantized matmul.

Reference computation (all shapes hardcoded):
    out[s,b,m,n] = sum_k (x[s,b,m,k] + 66)*0.03 * (y[b,k,n] - 160)*0.025
    x: [7, 8, 1024, 1024] f32 holding ints in [-128, 127]
    y: [8, 1024, 1024]    f32 holding ints in [0, 255]
    out: [7, 8, 1024, 1024] f32

Sharding: data-parallel over B=8 -> one batch element b per NeuronCore.
Core b gets x[:, b] and y[b]; no collectives needed.

Device kernel (per core), fp8 DoubleRow variant:
  - The rel-err gate is 2e-2; quantizing the zero-point-shifted operands
    (x+66 in [-62,193], y-160 in [-160,95]) to TRN fp8e4 (e4m3, max 240)
    costs 4.8e-3 rel err (validated in numpy AND on hw) -- well inside
    the gate. Host pre-applies the zero points during the fp8 cast, so
    the device does no dequant arithmetic at all; the combined scale
    0.03*0.025 = 7.5e-4 is fused into the PSUM->SBUF eviction.
  - fp8e4 matmuls in DoubleRow mode contract 256 k-elements per
    instruction (2 multiplies/cell/cycle): half the bf16 instruction
    count for the same work. 448 MMs x 213ns = 95.5us PE floor; the
    kernel streams them back-to-back at that rate (measured).
  - Plain DoubleRow ran MMs at 259ns: the 256-column non-contiguous
    LDWEIGHTS stole the rhs stream's SBUF/XBUS bandwidth. With
    DoubleRowSwInterleave the host pre-interleaves each weight tile
    into one contiguous 256B/partition block; LDWEIGHTS (130ns) then
    overlaps 100% and MMs hit the 213ns roofline. The y tiles must
    stay [128, 2, 1024] (i-row stride 1024B): per-(ki,nj) half tiles
    throttle every MM to 259ns (measured 20us slower).
  - Startup (trace-measured): the runtime preamble gates the first DMA
    issue to ~7.3us; the PE HAM clock gate holds 1.2GHz until its
    trailing activity window fills, and every PE idle gap pushes the
    2.4GHz ramp later. Dummy warm-up matmuls keep PE busy from the
    preamble to the first operand arrival (ramp ~11.5us vs ~17us with
    a gappy start). The first 1.5MB of operands are bandwidth-bound:
    y rides the sync HWDGE ring, the first head x chunk rides the
    scalar HWDGE ring (prompt completion events; its dma_start
    precedes the act-table load and first eviction in program order),
    and the rest of x rides the gpsimd SWDGE ring -- HWDGE+SWDGE pull
    in parallel, while two HWDGE rings would share one ~180GB/s
    budget (measured 4us slower). SWDGE completion EVENTS lag ~2-3us
    while the gpsimd sequencer still has dma_start issues queued, so
    the gpsimd descriptor list is kept short: 3 head chunks, ONE
    640KB tail transfer (p-major host layout), one 1MB descriptor per
    s>=1, with s>=3 additionally held back by pool recycling.
  - The s=0 head runs 4 output stripes ki-outer so each 128KB head
    chunk feeds ~1.7us of matmuls at the chunk arrival cadence.
  - Eviction alternates ScalarE/DVE per stripe and store issues ride
    the sync queue: one queue cannot hold 57 x 1.26us evictions plus
    57 x 0.7us dma_start issue slots inside the PE span.
  - Output is stored bf16 (halves out-DMA; +2e-4 rel err) and upcast
    to f32 on the host.
  - Tail: the final stripe's two half-evictions drain on both evictor
    engines and their store issues ride different queues (scalar +
    sync) so they don't serialize after the last matmul.
"""

import numpy as np
import ml_dtypes

import concourse.bass as bass
import concourse.mybir as mybir
from concourse import bacc
from concourse.tile import TileContext
from concourse.bass_utils import run_bass_kernel_spmd

S, B, M, K, N = 7, 8, 1024, 1024, 1024
P = 128          # SBUF partitions / PE array dim
NB = 512         # one PSUM bank of fp32
KP = 2 * P       # k-elements contracted per DoubleRow matmul
KTT, MTT = K // KP, M // P  # 4, 8 (host-side tiling of the x layout)
MJH = 4          # head stripes (s=0 warm-up path)
X_ZP = -66.0
Y_ZP = 160.0
OUT_SCALE = 0.03 * 0.025
BF16 = mybir.dt.bfloat16
FP8 = mybir.dt.float8e4
F32 = mybir.dt.float32
ACT_COPY = mybir.ActivationFunctionType.Copy
DR_SW = mybir.MatmulPerfMode.DoubleRowSwInterleave

_CACHED_NC = None


def build():
    # Bacc (not plain Bass): its finalize() runs generate_event_semaphores,
    # which splits multi-wait sync_info to the <=1-wait-per-instruction HW
    # limit (walrus rejects the unsplit form with "Too many sync waits").
    nc = bacc.Bacc("TRN2", target_bir_lowering=False)
    KT, MT, NT = K // KP, M // P, N // NB  # 4, 8, 2
    MR = MT - MJH  # tail stripes of s=0
    # x weight layout (DoubleRowSwInterleave, see _shard_inputs):
    # within a (ki2, mj) block of 256: position 2*j + i holds column
    # (127 - j) of k-subtile i. Three tensors so each startup DMA is a
    # single contiguous 2D transfer:
    #   x0h[ki, p, mj*256+c]           s=0, head stripes mj<MJH
    #   x0t[p, ki*MR*256 + mj'*256+c]  s=0, tail stripes (p-major: ONE DMA)
    #   xs[s-1, p, ki*MT*256 + mj*256+c]  s>=1 (p-major: one DMA per s)
    x0h_d = nc.declare_dram_parameter("x0h", [KT, P, MJH * 2 * P], FP8,
                                      isOutput=False)
    x0t_d = nc.declare_dram_parameter("x0t", [P, KT * MR * 2 * P], FP8,
                                      isOutput=False)
    xs_d = nc.declare_dram_parameter("xs", [S - 1, P, KT * MT * 2 * P], FP8,
                                     isOutput=False)
    # y pre-tiled per ki2: y_d[ki2, p, i, n] = yq[ki2*256+i*128+p, n]
    y_d = nc.declare_dram_parameter("y", [KT, P, 2, N], FP8, isOutput=False)
    o_d = nc.declare_dram_parameter("out", [S, M, N], BF16, isOutput=True)

    with TileContext(nc) as tc:
        with tc.tile_pool(name="ypool", bufs=1) as ypool, \
             tc.tile_pool(name="hpool", bufs=KT) as hpool, \
             tc.tile_pool(name="tpool", bufs=1) as tpool, \
             tc.tile_pool(name="xpool", bufs=2) as xpool, \
             tc.tile_pool(name="pspool", bufs=4, space="PSUM") as pspool, \
             tc.tile_pool(name="opool", bufs=6) as opool:
            # Warm-up: dummy matmuls keep PE busy from the preamble to the
            # first operand arrival (HAM clock ramp; see module docstring).
            # Only one column is memset (tile allocation needs a producer);
            # the rest is read as garbage, which is fine: the PE has no
            # traps, the warm PSUM bank is never read, and the first real
            # matmul's start=True resets it.
            warm_src = ypool.tile([P, NB], BF16, tag="warmsrc")
            nc.vector.memset(warm_src[:, 0:1], 1.0)
            warm_ps = pspool.tile([P, N], F32, tag="ps", name="warm")
            for _ in range(7):
                nc.tensor.matmul(warm_ps[:, 0:NB], warm_src[:, 0:P],
                                 warm_src[:], start=True, stop=True)
            nc.tensor.matmul(warm_ps[:, 0:NB // 2], warm_src[:, 0:P],
                             warm_src[:, 0:NB // 2], start=True, stop=True)

            # Startup loads (ring assignment rationale in module docstring).
            yq = [None] * KT
            xh = [None] * KT
            for ki in range(KT):
                xh[ki] = hpool.tile([P, MJH, 2 * P], FP8, tag="xh",
                                    name=f"xh{ki}")
                yq[ki] = ypool.tile([P, 2, N], FP8, tag=f"y{ki}",
                                    name=f"yt{ki}")
            xt0 = tpool.tile([P, KT, MR, 2 * P], FP8, tag="xt0")
            nc.scalar.dma_start(out=xh[0][:], in_=x0h_d[0])
            for ki in range(KT):
                nc.sync.dma_start(out=yq[ki][:], in_=y_d[ki])
            for ki in range(1, KT):
                nc.gpsimd.dma_start(out=xh[ki][:], in_=x0h_d[ki])
            nc.gpsimd.dma_start(out=xt0[:], in_=x0t_d[:])

            def evict(ot_sl, ps_sl, odd):
                # PSUM -> SBUF bf16 with fused scale, alternating between
                # the Scalar and Vector engines so neither eviction queue
                # accumulates backlog against the PE stream (a single queue
                # carrying all 57 x ~1.26us evictions plus issue overhead
                # runs within ~5% of the whole kernel span).
                if odd:
                    nc.vector.tensor_scalar_mul(ot_sl, ps_sl, OUT_SCALE)
                else:
                    nc.scalar.activation(ot_sl, ps_sl, ACT_COPY,
                                         scale=OUT_SCALE)

            def store(dram_sl, ot_sl, odd, queue=None):
                # store issues ride the near-idle sync queue: the ~0.7us
                # dma_start sequencer cost plus the ~0.75us cross-queue
                # wait fit easily there, and the store is off the
                # PSUM-recycle critical path (it only reads the SBUF copy)
                (queue or nc.sync).dma_start(out=dram_sl, in_=ot_sl)

            def mj_group(s, mj, lhsT_of, odd, split_evict=False):
                """One output stripe [128, 1024]: ki-inner accumulation into
                a 2-bank PSUM tile, then a single eviction + store. For the
                very last group, evict/store per nj half instead so the nj=0
                half drains while nj=1's final matmuls still stream."""
                pst = pspool.tile([P, N], F32, tag="ps", name="ps")
                ot = opool.tile([P, N], BF16, tag="o", name="ot")
                for ki in range(KT):
                    lhsT = lhsT_of(ki, mj)
                    for nj in range(NT):
                        nc.tensor.matmul(
                            pst[:, nj * NB:(nj + 1) * NB], lhsT,
                            yq[ki][:, :, nj * NB:(nj + 1) * NB],
                            start=(ki == 0), stop=(ki == KT - 1),
                            perf_mode=DR_SW)
                if split_evict:
                    # last stripe: drain the two nj halves on the two
                    # evictor queues in parallel, store issues on
                    # different queues so they don't serialize
                    for nj in range(NT):
                        sl = slice(nj * NB, (nj + 1) * NB)
                        evict(ot[:, sl], pst[:, sl], nj % 2)
                        store(o_d[s, mj * P:(mj + 1) * P, sl], ot[:, sl],
                              nj % 2,
                              queue=(nc.scalar if nj == 0 else nc.sync))
                else:
                    evict(ot[:], pst[:], odd)
                    store(o_d[s, mj * P:(mj + 1) * P, :], ot[:], odd)

            # s=0: head stripes ki-outer (consume each head chunk as it
            # lands), then the tail stripes from the single tail transfer.
            head = [pspool.tile([P, N], F32, tag="ps", name=f"ph{mj}")
                    for mj in range(MJH)]
            for ki in range(KT):
                for mj in range(MJH):
                    lhsT = xh[ki][:, mj, :]
                    for nj in range(NT):
                        nc.tensor.matmul(
                            head[mj][:, nj * NB:(nj + 1) * NB], lhsT,
                            yq[ki][:, :, nj * NB:(nj + 1) * NB],
                            start=(ki == 0), stop=(ki == KT - 1),
                            perf_mode=DR_SW)
            for mj in range(MJH):
                ot = opool.tile([P, N], BF16, tag="o", name="oth")
                evict(ot[:], head[mj][:], mj % 2)
                store(o_d[0, mj * P:(mj + 1) * P, :], ot[:], mj % 2)
            for mj in range(MJH, MT):
                mj_group(0, mj, lambda ki, mj: xt0[:, ki, mj - MJH, :],
                         mj % 2)

            # s>=1: one 1MB descriptor per s on the gpsimd ring; xpool
            # bufs=2 lets s+1 prefetch while s streams and holds s+2 back
            # until s's last LDWEIGHTS.
            for s in range(1, S):
                xb = xpool.tile([P, KT, MT, 2 * P], FP8, tag="xb")
                nc.gpsimd.dma_start(out=xb[:], in_=xs_d[s - 1])
                for mj in range(MT):
                    mj_group(s, mj, lambda ki, mj: xb[:, ki, mj, :],
                             mj % 2,
                             split_evict=(s == S - 1 and mj == MT - 1))

            # Tail warm-down: exec time is measured to the END of the
            # runtime's semaphore-reset postamble, and the HAM clock
            # drops to 1.2GHz ~5us after the last PE activity -- which
            # would put the whole postamble at half clock. Dummy matmuls
            # during the final eviction/store drain (they recycle a PSUM
            # buffer whose eviction completed long ago, and Tensor would
            # otherwise sit idle before the end barrier) keep the clock
            # high through most of the postamble.
            tail_ps = pspool.tile([P, N], F32, tag="ps", name="tailps")
            for _ in range(12):
                nc.tensor.matmul(tail_ps[:, 0:NB], warm_src[:, 0:P],
                                 warm_src[:], start=True, stop=True)
    nc.finalize()
    return nc


def _shard_inputs(x, y):
    f8 = ml_dtypes.float8_e4m3
    in_maps = []
    MR = MTT - MJH
    for b in range(B):
        # zero points pre-applied; |values| <= 193 fit e4m3 (max 240)
        # with <= 6.25% per-element rounding error -> ~4.6e-3 rel err.
        # x shard: k-major transpose, then the DoubleRowSwInterleave weight
        # layout (see build()): per (s, ki2, mj) block of 256, position
        # 2*j + i holds column (127 - j) of k-subtile i.
        xq = (np.ascontiguousarray(x[:, b].transpose(0, 2, 1))
              - np.float32(X_ZP)).astype(f8)          # [S, K, M]
        a = xq.reshape(S, KTT, 2, P, MTT, P)          # [s, ki2, i, p, mj, j]
        a = a.transpose(0, 1, 3, 4, 5, 2)[:, :, :, :, ::-1, :]
        a = np.ascontiguousarray(a).reshape(S, KTT, P, MTT * 2 * P)
        x0h = np.ascontiguousarray(a[0][:, :, :MJH * 2 * P])
        x0t = np.ascontiguousarray(
            a[0][:, :, MJH * 2 * P:].transpose(1, 0, 2)).reshape(
                P, KTT * MR * 2 * P)
        xs = np.ascontiguousarray(a[1:].transpose(0, 2, 1, 3)).reshape(
            S - 1, P, KTT * MTT * 2 * P)
        # y: per-ki2 DoubleRow tile layout [ki2, p, i, n] (one DMA per tile)
        yq = (y[b] - np.float32(Y_ZP)).astype(f8)    # [K, N]
        yq = yq.reshape(KTT, 2, P, N).transpose(0, 2, 1, 3)
        in_maps.append({
            "x0h": x0h,
            "x0t": x0t,
            "xs": xs,
            "y": np.ascontiguousarray(yq),
        })
    return in_maps


def run(x, y, trace=False):
    global _CACHED_NC
    if _CACHED_NC is None:
        _CACHED_NC = build()
    nc = _CACHED_NC
    in_maps = _shard_inputs(x, y)
    res = run_bass_kernel_spmd(nc, in_maps, core_ids=list(range(B)), trace=trace)
    out = np.stack([np.asarray(res.results[b]["out"]) for b in range(B)], axis=1)
    return out.astype(np.float32), res


def kernel(x, y):
    out, _ = run(x, y, trace=False)
    return out
